# revision 13
# baseline (speedup 1.0000x reference)
"""Multi-head attention (B=2, S=2048, D=1024, H=16, dk=64) on 8 trn2 cores.

Sharding: core c handles batch b=c//4 and 4 heads g=c%4 (heads 4g..4g+3).
Each core computes its heads' Q/K/V projections, attention, and a partial
output projection; the host sums the 4 partials per batch.

Per-core layout — everything k-major so softmax feeds the PV matmul with
no transpose:
  - host pre-transposes x -> xT [D, S]; projections contract over D.
  - qhT/khT [128 head-dims (2 heads), S] bf16.
  - scoresT[k, q] = khT.T @ qhT per 128-key tile into a 6-bank PSUM ring;
    ScalarE exponentiates ring batches (3,3,3,3,4 kt) straight into SBUF
    bf16 tiles that are the PV moving operand.
  - PV: ctx[dk+sums, q] += [vh | ones].T @ exp. The 64 all-ones weight
    columns make PSUM partitions 64-127 the softmax denominators
    (replicated), so normalization is one reciprocal + one tensor_mul
    with partition-offset operands during the PSUM->SBUF ctx copy.
  - no max-subtraction (scores ~ N(0,1), fp32-safe); bq added via
    per-partition tensor_scalar during the qhT copy (scaled by
    1/sqrt(dk) on host, folded into wq/bq); bk is softmax-invariant and
    dropped; bv/bo fold into a host-side correction.
  - out proj per 512-q group overlaps the next group's attention;
    partials stored bf16.
"""

import sys

for _p in ("/opt/trn_rl_repo",):
    if _p not in sys.path:
        sys.path.insert(0, _p)

from contextlib import ExitStack

import ml_dtypes
import numpy as np

import concourse.bass as bass
import concourse.bacc as bacc_mod
import concourse.mybir as mybir
import concourse.tile as tile
from concourse.bass_utils import run_bass_kernel_spmd

BF16 = mybir.dt.bfloat16
F32 = mybir.dt.float32
AF = mybir.ActivationFunctionType

B, S, D = 2, 2048, 1024
NCORES = 8
HLOC = 4          # heads per core
DK = 64
HD = HLOC * DK    # local head dims = 256
KT_D = D // 128   # 8 k-tiles over the model dim
NST = S // 128    # 16 tiles over sequence (key tiles)
NQG = 4           # q groups of 512
RING = 6          # scores PSUM ring slots (banks)
EXP_BATCH = (3, 3, 3, 3, 4)   # kt per activation; slots stay contiguous


def build_nc() -> bass.Bass:
    nc = bacc_mod.Bacc()

    xqT = nc.dram_tensor("xqT", [D, S], BF16, kind="ExternalInput")
    xkT = nc.dram_tensor("xkT", [D, S], BF16, kind="ExternalInput")
    xvT = nc.dram_tensor("xvT", [D, S], BF16, kind="ExternalInput")
    wqT = nc.dram_tensor("wqT", [D, HD], BF16, kind="ExternalInput")
    wkT = nc.dram_tensor("wkT", [D, HD], BF16, kind="ExternalInput")
    wvT = nc.dram_tensor("wvT", [D, HD], BF16, kind="ExternalInput")
    woT = nc.dram_tensor("woT", [HD, D], BF16, kind="ExternalInput")
    bqd = nc.dram_tensor("bq", [HD], F32, kind="ExternalInput")
    out = nc.dram_tensor("out_partial", [S, D], BF16, kind="ExternalOutput")

    with tile.TileContext(nc) as tc, ExitStack() as ctx:
        const = ctx.enter_context(tc.tile_pool(name="const", bufs=1))
        persist = ctx.enter_context(tc.tile_pool(name="persist", bufs=1))

        # Weights / bias resident in SBUF.
        wq_s = const.tile([128, KT_D, HD], BF16, tag="wq")
        wk_s = const.tile([128, KT_D, HD], BF16, tag="wk")
        wv_s = const.tile([128, KT_D, HD], BF16, tag="wv")
        wo_s = const.tile([128, 2, D], BF16, tag="wo")
        bq_s = const.tile([128, 2], F32, tag="bq")
        nc.sync.dma_start(wq_s, wqT.rearrange("(t p) n -> p t n", p=128))
        nc.sync.dma_start(wk_s, wkT.rearrange("(t p) n -> p t n", p=128))
        nc.sync.dma_start(wv_s, wvT.rearrange("(t p) n -> p t n", p=128))
        nc.sync.dma_start(wo_s, woT.rearrange("(t p) n -> p t n", p=128))
        nc.sync.dma_start(bq_s, bqd.rearrange("(m p) -> p m", p=128))

        # Projection outputs resident in SBUF. vh_aug holds [vh | ones]
        # per (kt, head): cols 64-127 all-ones make the PV matmul emit
        # softmax denominators into PSUM partitions 64-127.
        qhT = [persist.tile([128, S], BF16, tag=f"qhT{m}", name=f"qhT{m}") for m in range(2)]
        khT = [persist.tile([128, S], BF16, tag=f"khT{m}", name=f"khT{m}") for m in range(2)]
        vh_aug = persist.tile([128, NST * HLOC, 128], BF16, tag="vh")
        nc.vector.memset(vh_aug[:, :, 64:128], 1.0)

        # ---- x loads: chunks round-robin over four DMA queues ------
        xpool = ctx.enter_context(tc.tile_pool(name="xload", bufs=2))
        QS = (nc.gpsimd, nc.sync, nc.scalar)
        xk_t = xpool.tile([128, KT_D, S], BF16, tag="x", name="xk_t")
        for kt in range(KT_D):
            QS[kt % 3].dma_start(xk_t[:, kt], xkT[kt * 128 : (kt + 1) * 128, :])
        xq_t = xpool.tile([128, KT_D, S], BF16, tag="x", name="xq_t")
        for kt in range(KT_D):
            QS[kt % 3].dma_start(xq_t[:, kt], xqT[kt * 128 : (kt + 1) * 128, :])
        xv_t = xpool.tile([128, KT_D, S], BF16, tag="x", name="xv_t")
        for kt in range(KT_D):
            QS[kt % 3].dma_start(xv_t[:, kt], xvT[kt * 128 : (kt + 1) * 128, :])

        # ---- Phase 1a: K then Q projections ------------------------
        with tc.tile_pool(name="qk_psum", bufs=2, space="PSUM") as qkp:
            for which, x_t, w_s in (("k", xk_t, wk_s), ("q", xq_t, wq_s)):
                for m in range(2):
                    ps = qkp.tile([128, S], F32, tag="proj", name=f"ps_{which}{m}")
                    for kt in range(KT_D):
                        for nb in range(4):
                            nc.tensor.matmul(
                                ps[:, nb * 512 : (nb + 1) * 512],
                                lhsT=w_s[:, kt, m * 128 : (m + 1) * 128],
                                rhs=x_t[:, kt, nb * 512 : (nb + 1) * 512],
                                start=(kt == 0),
                                stop=(kt == KT_D - 1),
                            )
                    if which == "k":
                        nc.vector.tensor_copy(khT[m], ps)
                    else:
                        nc.vector.tensor_scalar_add(qhT[m], ps, bq_s[:, m : m + 1])

        # ---- Phase 1b: V projection (into vh_aug cols 0-63) --------
        with tc.tile_pool(name="v_psum", bufs=4, space="PSUM") as vps:
            for st in range(NST):
                ps = vps.tile([128, HD], F32, tag="vproj")
                for kt in range(KT_D):
                    nc.tensor.matmul(
                        ps,
                        lhsT=xv_t[:, kt, st * 128 : (st + 1) * 128],
                        rhs=wv_s[:, kt, :],
                        start=(kt == 0),
                        stop=(kt == KT_D - 1),
                    )
                nc.vector.tensor_copy(
                    vh_aug[:, st * HLOC : (st + 1) * HLOC, 0:64],
                    ps.rearrange("p (h d) -> p h d", h=HLOC),
                )

        # ---- Phase 2: attention, cross-head software pipeline ------
        # Section i runs head i's scores+exp interleaved 1:1 with head
        # i-1's PV (whose exp tile finished a full section ago), so the
        # PE's only long wait is one consolidated ring-WAR bubble per
        # section boundary instead of one per exp batch.
        batches = []
        _k0 = 0
        for _bsz in EXP_BATCH:
            batches.append((_k0, _bsz))
            _k0 += _bsz
        NB = len(batches)

        ctxT_all = []  # (qg, hp) -> tile, consumed by the out-proj fillers
        with (
            tc.tile_pool(name="ring_psum", bufs=1, space="PSUM") as ringp,
            tc.tile_pool(name="acc_psum", bufs=2, space="PSUM") as accp,
            tc.tile_pool(name="exp_pool", bufs=2) as epool,
            tc.tile_pool(name="ctxT_pool", bufs=4) as ctpool,
            tc.tile_pool(name="recip_pool", bufs=2) as rpool,
            tc.tile_pool(name="out_sbuf", bufs=3) as outs,
        ):
            ring = ringp.tile([128, RING, 512], F32, tag="ring")

            class Sec:
                pass

            def emit_scores(s, b):
                k0, bsz = batches[b]
                for kt in range(k0, k0 + bsz):
                    nc.tensor.matmul(
                        ring[:, kt % RING, :],
                        lhsT=khT[s.hp][s.po : s.po + 64, kt * 128 : (kt + 1) * 128],
                        rhs=qhT[s.hp][s.po : s.po + 64, s.qs],
                        start=True,
                        stop=True,
                    )

            def emit_exp(s, b):
                k0, bsz = batches[b]
                nc.scalar.activation(
                    s.exp_t[:, k0 : k0 + bsz, :],
                    ring[:, k0 % RING : k0 % RING + bsz, :],
                    AF.Exp,
                )

            def emit_pv(s, b):
                k0, bsz = batches[b]
                for kt in range(k0, k0 + bsz):
                    nc.tensor.matmul(
                        s.ctx_ps,
                        lhsT=vh_aug[:, kt * HLOC + s.h, :],
                        rhs=s.exp_t[:, kt, :],
                        start=(kt == 0),
                        stop=(kt == NST - 1),
                    )

            def emit_normalize(s):
                rec = rpool.tile([128, 512], F32, tag="recip")
                nc.vector.reciprocal(rec[64:128, :], s.ctx_ps[64:128, :])
                nc.vector.tensor_mul(
                    s.ctxT[s.po : s.po + 64, :], s.ctx_ps[0:64, :], rec[64:128, :]
                )

            # Out-proj st jobs run as PE fillers inside later sections,
            # accumulating in ring slots 4/5 (idle between exp batches).
            out_jobs = []

            def emit_out_job():
                qg, i = out_jobs.pop(0)
                ctxT = ctxT_all[qg]
                st = qg * 4 + i
                ob = outs.tile([128, D], BF16, tag="ob")
                for nb in range(2):
                    ops = ring[:, 4 + nb, :]
                    for kt in range(2):
                        nc.tensor.matmul(
                            ops,
                            lhsT=ctxT[kt][:, i * 128 : (i + 1) * 128],
                            rhs=wo_s[:, kt, nb * 512 : (nb + 1) * 512],
                            start=(kt == 0),
                            stop=(kt == 1),
                        )
                    nc.vector.tensor_copy(ob[:, nb * 512 : (nb + 1) * 512], ops)
                nc.gpsimd.dma_start(out[st * 128 : (st + 1) * 128, :], ob)

            prev = None
            for sec_i in range(NQG * HLOC):
                qg, h = divmod(sec_i, HLOC)
                s = Sec()
                s.h = h
                s.qg = qg
                s.hp, s.po = divmod(h, 2)
                s.po *= 64
                s.qs = slice(qg * 512, (qg + 1) * 512)
                if h == 0:
                    s.ctxT_pair = [
                        ctpool.tile([128, 512], BF16, tag=f"ctxT{hp}", name=f"ctxT{hp}")
                        for hp in range(2)
                    ]
                    ctxT_all.append(s.ctxT_pair)
                else:
                    s.ctxT_pair = prev.ctxT_pair
                s.ctxT = s.ctxT_pair[s.hp]
                s.exp_t = epool.tile([128, NST, 512], BF16, tag="exp")
                s.ctx_ps = accp.tile([128, 512], F32, tag="acc", name="ctx_ps")

                # PE order: front-load prev's PV (deps one section old) so
                # the PE has a single long runnable stretch; scores batches
                # land just-in-time behind their exp ring-WAR deps.
                emit_scores(s, 0)
                emit_exp(s, 0)
                emit_scores(s, 1)
                emit_exp(s, 1)
                if prev is not None:
                    emit_pv(prev, 0)
                    emit_pv(prev, 1)
                    emit_pv(prev, 2)
                emit_scores(s, 2)
                emit_exp(s, 2)
                if prev is not None:
                    emit_pv(prev, 3)
                emit_scores(s, 3)
                emit_exp(s, 3)
                if prev is not None:
                    emit_pv(prev, 4)
                emit_scores(s, 4)
                emit_exp(s, 4)
                if prev is not None:
                    emit_normalize(prev)
                    if prev.h == HLOC - 1:
                        # prev closed a q-group: its 4 out-proj jobs are ready
                        out_jobs.extend((prev.qg, i) for i in range(4))
                if out_jobs:
                    emit_out_job()
                prev = s
            for b in range(NB):
                emit_pv(prev, b)
            emit_normalize(prev)
            out_jobs.extend((prev.qg, i) for i in range(4))
            while out_jobs:
                emit_out_job()

    nc.compile()
    return nc


_CACHE: dict = {}


def _get_nc() -> bass.Bass:
    if "nc" not in _CACHE:
        _CACHE["nc"] = build_nc()
    return _CACHE["nc"]


def _bf16(x: np.ndarray) -> np.ndarray:
    return np.ascontiguousarray(x).astype(ml_dtypes.bfloat16)


def make_in_maps(q, k, v, wq, bq, wk, bk, wv, bv, wo, bo):
    scale = np.float32(1.0 / np.sqrt(DK))
    in_maps = []
    for c in range(NCORES):
        b, g = divmod(c, 4)
        hh = g * HD
        in_maps.append(
            {
                "xqT": _bf16(np.asarray(q[b], np.float32).T),
                "xkT": _bf16(np.asarray(k[b], np.float32).T),
                "xvT": _bf16(np.asarray(v[b], np.float32).T),
                "wqT": _bf16(np.asarray(wq[hh : hh + HD], np.float32).T * scale),
                "wkT": _bf16(np.asarray(wk[hh : hh + HD], np.float32).T),
                "wvT": _bf16(np.asarray(wv[hh : hh + HD], np.float32).T),
                "woT": _bf16(np.asarray(wo[:, hh : hh + HD], np.float32).T),
                "bq": np.ascontiguousarray(np.asarray(bq[hh : hh + HD], np.float32) * scale),
            }
        )
    return in_maps


def assemble(results, bv, bo, wo) -> np.ndarray:
    out = np.zeros((B, S, D), np.float32)
    for c in range(NCORES):
        out[c // 4] += np.asarray(results[c]["out_partial"], np.float32)
    corr = np.asarray(bv, np.float32) @ np.asarray(wo, np.float32).T + np.asarray(
        bo, np.float32
    )
    out += corr[None, None, :]
    return out


def kernel(q, k, v, wq, bq, wk, bk, wv, bv, wo, bo) -> np.ndarray:
    nc = _get_nc()
    in_maps = make_in_maps(q, k, v, wq, bq, wk, bk, wv, bv, wo, bo)
    res = run_bass_kernel_spmd(nc, in_maps, list(range(NCORES))).results
    return assemble(res, bv, bo, wo)
